# revision 5
# baseline (speedup 1.0000x reference)
"""Distributed Trainium2 Bass kernel for causal multi-head attention.

Problem: B=2, T=2048, C=1024, H=16 heads (Dh=64), RoPE + causal mask +
softmax + output projection.

Sharding: 8 cores = batch (2) x head-groups (4 heads each). Each core
computes q/k/v projections for its 4 heads, RoPE, attention, and a
partial output projection y_partial = out_heads @ Wo_slice.T. The host
sums the 4 partials per batch element.

Layout trick: everything is computed in "head-dim-major" (transposed)
layout so no on-chip transposes are needed:
  qT/kT: (dims, tokens) from projection matmuls directly
  S^T = K @ Q^T tiles (keys, tokens): softmax denominator via an
    appended ones-column in V (extra row of PV output = sum over keys)
  PV: O'^T = V_aug^T @ P^T -> (65, toks) in PSUM, row 64 = denominator
  o-proj consumes O^T directly as the stationary operand.
"""

import os
import sys
import types
import numpy as np

sys.path.insert(0, "/opt/trn_rl_repo")

import ml_dtypes
import concourse.bass as bass
import concourse.mybir as mybir
from concourse import bacc
from concourse.tile import TileContext
from concourse import bass_utils
from concourse.bass import ts, ds

F32 = mybir.dt.float32
BF16 = mybir.dt.bfloat16

B, T, C, H = 2, 2048, 1024, 16
Dh = C // H          # 64
HG = 4               # heads per core
NCORES = 8
KC = C // 128        # 8 contraction tiles for projections
NCHUNK = T // 512    # 4 token chunks
KT = T // 128        # 16 key tiles
SCALE = Dh ** -0.5   # 0.125


def _install_ntff_hook():
    """The NTFF profiling hook module is absent in this image; inject it."""
    if "antenv.axon_hooks" in sys.modules:
        return
    try:
        import trn_agent_boot.trn_boot as tb
        mod = types.ModuleType("antenv.axon_hooks")
        hook = tb._ntff_profile_via_ctypes("/opt/axon/libaxon_pjrt.so")
        mod.get_axon_ntff_profile_hook = lambda: hook
        sys.modules["antenv.axon_hooks"] = mod
    except Exception:
        pass


def build(mode: str) -> bass.Bass:
    """mode: 'causal' | 'full' | 'general'"""
    assert mode in ("causal", "full", "general")
    nc = bacc.Bacc(None, target_bir_lowering=False)

    xT = nc.dram_tensor("xT", [C, T], F32, kind="ExternalInput")
    wq = nc.dram_tensor("wq", [C, 256], F32, kind="ExternalInput")
    wk = nc.dram_tensor("wk", [C, 256], F32, kind="ExternalInput")
    wv = nc.dram_tensor("wv", [C, 256], F32, kind="ExternalInput")
    wo = nc.dram_tensor("wo", [256, C], BF16, kind="ExternalInput")
    cos2 = nc.dram_tensor("cos2", [128, T], F32, kind="ExternalInput")
    sin2 = nc.dram_tensor("sin2", [128, T], F32, kind="ExternalInput")
    tri = nc.dram_tensor("tri", [128, 128], BF16, kind="ExternalInput")
    if mode == "general":
        maskT = nc.dram_tensor("maskT", [T, T], BF16, kind="ExternalInput")
    y = nc.dram_tensor("out", [T, C], F32, kind="ExternalOutput")

    with TileContext(nc) as tc:
        with (
            tc.tile_pool(name="persist", bufs=1) as persist,
            tc.tile_pool(name="work", bufs=2) as work,
            tc.tile_pool(name="epool", bufs=3) as epool,
        ):
            # ---- persistent SBUF tensors (live through both phases) ----
            qT_sb = [persist.tile([128, T], BF16, name=f"qT{p}") for p in range(2)]
            kT_sb = [persist.tile([128, T], BF16, name=f"kT{p}") for p in range(2)]
            # v token-major with interleaved ones column per head: 4 x 65 cols
            v_sb = [persist.tile([128, HG * (Dh + 1)], BF16, name=f"v{j}")
                    for j in range(KT)]
            wo_sb = persist.tile([128, 2, C], BF16, name="wo_sb")
            tri_sb = persist.tile([128, 128], BF16, name="tri_sb")
            nc.sync.dma_start(wo_sb[:], wo.rearrange("(p2 p) n -> p p2 n", p=128))
            nc.sync.dma_start(tri_sb[:], tri[:])

            # ================= phase 1: projections + RoPE =================
            with (
                tc.tile_pool(name="xw", bufs=1) as xw,
                tc.tile_pool(name="ppsum", bufs=4, space="PSUM") as ppsum,
                tc.tile_pool(name="rope", bufs=3) as rope,
            ):
                x_sb = xw.tile([128, KC, T], F32)
                xv = xT.rearrange("(kt p) t -> kt p t", p=128)
                for k in range(KC):
                    nc.sync.dma_start(x_sb[:, k, :], xv[k])
                w_sb = {}
                for nm, dram in (("q", wq), ("k", wk), ("v", wv)):
                    w_sb[nm] = xw.tile([128, KC, 256], F32, name=f"w{nm}_sb")
                    nc.sync.dma_start(
                        w_sb[nm][:], dram.rearrange("(kt p) m -> p kt m", p=128))
                cos_sb = xw.tile([128, T], F32)
                sin_sb = xw.tile([128, T], F32)
                nc.sync.dma_start(cos_sb[:], cos2[:])
                nc.sync.dma_start(sin_sb[:], sin2[:])

                # q/k projections -> head-dim-major + fused RoPE
                for nm, dest in (("q", qT_sb), ("k", kT_sb)):
                    for p in range(2):          # head-pair tile (2 heads)
                        for n in range(NCHUNK):  # 512-token chunk
                            ps = ppsum.tile([128, 512], F32, tag="proj")
                            for k in range(KC):
                                nc.tensor.matmul(
                                    ps[:],
                                    w_sb[nm][:, k, ts(p, 128)],
                                    x_sb[:, k, ts(n, 512)],
                                    start=(k == 0), stop=(k == KC - 1))
                            # RoPE: out = ps*cos + swap32(ps * sin_preswapped)
                            t1 = rope.tile([128, 512], F32, tag="t1")
                            nc.vector.tensor_mul(
                                t1[:], ps[:], cos_sb[:, ts(n, 512)])
                            t2p = rope.tile([128, 512], F32, tag="t2p")
                            nc.vector.tensor_mul(
                                t2p[:], ps[:], sin_sb[:, ts(n, 512)])
                            t2 = rope.tile([128, 512], F32, tag="t2")
                            for a, bq in ((0, 32), (32, 0), (64, 96), (96, 64)):
                                nc.sync.dma_start(
                                    t2[a:a + 32, :], t2p[bq:bq + 32, :])
                            nc.vector.tensor_add(
                                dest[p][:, ts(n, 512)], t1[:], t2[:])

                # v projection: token-major with ones column per head
                for tt in range(T // 128):
                    ps = ppsum.tile([128, 256], F32, tag="vproj")
                    for k in range(KC):
                        nc.tensor.matmul(
                            ps[:],
                            x_sb[:, k, ts(tt, 128)],
                            w_sb["v"][:, k, :],
                            start=(k == 0), stop=(k == KC - 1))
                    vt = v_sb[tt].rearrange("p (h d) -> p h d", h=HG)
                    nc.vector.tensor_copy(vt[:, :, 0:Dh], ps.rearrange(
                        "p (h d) -> p h d", h=HG))
                    nc.vector.memset(vt[:, :, Dh:Dh + 1], 1.0)

            # ================= phase 2: attention + o-proj =================
            with (
                tc.tile_pool(name="apsum", bufs=2, space="PSUM") as apsum,
                tc.tile_pool(name="opool", bufs=2) as opool,
                tc.tile_pool(name="mpool", bufs=1) as mpool,
            ):
                if mode == "general":
                    msk_sb = mpool.tile([128, KT, T], BF16)
                    mv = maskT.rearrange("(kt p) t -> kt p t", p=128)
                    for j in range(KT):
                        nc.sync.dma_start(msk_sb[:, j, :], mv[j])

                for c in range(NCHUNK):
                    o_sb = [opool.tile([128, 512], BF16, tag=f"o{p}",
                                       name=f"o_sb{p}") for p in range(2)]
                    for h in range(HG):
                        p, off = h // 2, (h % 2) * 64
                        nkt = 4 * (c + 1) if mode == "causal" else KT
                        ngroups = (nkt + 1) // 2
                        psO = apsum.tile([128, 512], F32, tag="o")
                        for g in range(ngroups):
                            psS = apsum.tile([128, 1024], F32, tag="s")
                            E = epool.tile([128, 1024], BF16, tag="E")
                            slots = [j for j in (2 * g, 2 * g + 1) if j < nkt]
                            for s_i, j in enumerate(slots):
                                n_off = 0
                                if mode == "causal" and j >= 4 * c:
                                    n_off = 128 * (j - 4 * c)
                                nc.tensor.matmul(
                                    psS[:, ds(512 * s_i + n_off, 512 - n_off)],
                                    kT_sb[p][off:off + 64, ts(j, 128)],
                                    qT_sb[p][off:off + 64,
                                             ds(512 * c + n_off, 512 - n_off)],
                                    start=True, stop=True)
                            nc.scalar.activation(
                                E[:], psS[:],
                                mybir.ActivationFunctionType.Exp, scale=SCALE)
                            for s_i, j in enumerate(slots):
                                n_off = 0
                                if mode == "causal" and j >= 4 * c:
                                    n_off = 128 * (j - 4 * c)
                                    if n_off < 512:
                                        nc.vector.tensor_mul(
                                            E[:, ds(512 * s_i + n_off, 128)],
                                            E[:, ds(512 * s_i + n_off, 128)],
                                            tri_sb[:])
                                if mode == "general":
                                    nc.vector.tensor_mul(
                                        E[:, ts(s_i, 512)], E[:, ts(s_i, 512)],
                                        msk_sb[:, j, ts(c, 512)])
                                nc.tensor.matmul(
                                    psO[0:65, ds(n_off, 512 - n_off)],
                                    v_sb[j][:, ds(h * (Dh + 1), Dh + 1)],
                                    E[:, ds(512 * s_i + n_off, 512 - n_off)],
                                    start=(j == 0), stop=(j == nkt - 1))
                        # normalize: rows/denominator
                        r_sb = opool.tile([1, 512], F32, tag="r")
                        nc.vector.reciprocal(r_sb[:], psO[64:65, :])
                        rb_sb = opool.tile([64, 512], F32, tag="rb")
                        nc.gpsimd.partition_broadcast(rb_sb[:], r_sb[:])
                        nc.vector.tensor_mul(
                            o_sb[p][off:off + 64, :], psO[0:64, :], rb_sb[:])
                    # output projection for this chunk
                    for tt in range(4):
                        y_sb = opool.tile([128, C], F32, tag="y")
                        for nn in range(2):
                            psY = apsum.tile([128, 512], F32, tag="s")
                            for p in range(2):
                                nc.tensor.matmul(
                                    psY[:],
                                    o_sb[p][:, ts(tt, 128)],
                                    wo_sb[:, p, ts(nn, 512)],
                                    start=(p == 0), stop=(p == 1))
                            nc.vector.tensor_copy(y_sb[:, ts(nn, 512)], psY[:])
                        nc.sync.dma_start(y[ds(512 * c + 128 * tt, 128), :], y_sb[:])

    nc.finalize()
    return nc


_CACHE: dict = {}


def _get_nc(mode: str):
    if mode not in _CACHE:
        _CACHE[mode] = build(mode)
    return _CACHE[mode]


def kernel(x, cos, sin, mask, n_heads, Wq, Wk, Wv, Wo, _trace=False):
    _install_ntff_hook()
    x = np.asarray(x, np.float32)
    cos = np.asarray(cos, np.float32)
    sin = np.asarray(sin, np.float32)
    mask = np.asarray(mask)
    Wq = np.asarray(Wq, np.float32)
    Wk = np.asarray(Wk, np.float32)
    Wv = np.asarray(Wv, np.float32)
    Wo = np.asarray(Wo, np.float32)

    if np.array_equal(mask, np.tril(np.ones((T, T), mask.dtype))):
        mode = "causal"
    elif np.all(mask == 1):
        mode = "full"
    else:
        mode = "general"

    cosT = np.ascontiguousarray(cos.T)          # (64, T)
    # pre-swapped signed sin: after multiplying q by this and swapping the
    # 32-row halves, we get rotate_half(q)*sin in standard orientation.
    sinS = np.ascontiguousarray(sin.T).copy()
    sinS[32:64] *= -1.0
    cos2 = np.vstack([cosT, cosT])              # (128, T)
    sin2 = np.vstack([sinS, sinS])
    tri = np.triu(np.ones((128, 128), np.float32)).astype(ml_dtypes.bfloat16)

    in_maps = []
    for core in range(NCORES):
        b, g = core // 4, core % 4
        rows = slice(g * 256, (g + 1) * 256)
        m = {
            "xT": np.ascontiguousarray(x[b].T),
            "wq": np.ascontiguousarray(Wq[rows].T),
            "wk": np.ascontiguousarray(Wk[rows].T),
            "wv": np.ascontiguousarray(Wv[rows].T),
            "wo": np.ascontiguousarray(Wo[:, rows].T).astype(ml_dtypes.bfloat16),
            "cos2": cos2, "sin2": sin2, "tri": tri,
        }
        if mode == "general":
            m["maskT"] = np.ascontiguousarray(mask.T).astype(ml_dtypes.bfloat16)
        in_maps.append(m)

    nc = _get_nc(mode)
    res = bass_utils.run_bass_kernel_spmd(
        nc, in_maps, core_ids=list(range(NCORES)), trace=_trace)
    if _trace:
        kernel.last_result = res

    y = np.zeros((B, T, C), np.float32)
    for core in range(NCORES):
        y[core // 4] += res.results[core]["out"]
    return y


# revision 6
# speedup vs baseline: 1.6384x; 1.6384x over previous
"""Distributed Trainium2 Bass kernel for causal multi-head attention.

Problem: B=2, T=2048, C=1024, H=16 heads (Dh=64), RoPE + causal mask +
softmax + output projection.

Sharding: 8 cores = batch (2) x head-groups (4 heads each). Each core
computes q/k/v projections for its 4 heads, RoPE, attention, and a
partial output projection y_partial = out_heads @ Wo_slice.T. The host
sums the 4 partials per batch element.

Layout trick: everything is computed in "head-dim-major" (transposed)
layout so no on-chip transposes are needed:
  qT/kT: (dims, tokens) from projection matmuls directly
  S^T = K @ Q^T tiles (keys, tokens): softmax denominator via an
    appended ones-column in V (extra row of PV output = sum over keys)
  PV: O'^T = V_aug^T @ P^T -> (65, toks) in PSUM, row 64 = denominator
  o-proj consumes O^T directly as the stationary operand.
"""

import os
import sys
import types
import numpy as np

sys.path.insert(0, "/opt/trn_rl_repo")

import ml_dtypes
import concourse.bass as bass
import concourse.mybir as mybir
from concourse import bacc
from concourse.tile import TileContext
from concourse import bass_utils
from concourse.bass import ts, ds

F32 = mybir.dt.float32
BF16 = mybir.dt.bfloat16

B, T, C, H = 2, 2048, 1024, 16
Dh = C // H          # 64
HG = 4               # heads per core
NCORES = 8
KC = C // 128        # 8 contraction tiles for projections
NCHUNK = T // 512    # 4 token chunks
KT = T // 128        # 16 key tiles
SCALE = Dh ** -0.5   # 0.125


def _install_ntff_hook():
    """The NTFF profiling hook module is absent in this image; inject it."""
    if "antenv.axon_hooks" in sys.modules:
        return
    try:
        import trn_agent_boot.trn_boot as tb
        mod = types.ModuleType("antenv.axon_hooks")
        hook = tb._ntff_profile_via_ctypes("/opt/axon/libaxon_pjrt.so")
        mod.get_axon_ntff_profile_hook = lambda: hook
        sys.modules["antenv.axon_hooks"] = mod
    except Exception:
        pass


def build(mode: str) -> bass.Bass:
    """mode: 'causal' | 'full' | 'general'"""
    assert mode in ("causal", "full", "general")
    nc = bacc.Bacc(None, target_bir_lowering=False)

    xT = nc.dram_tensor("xT", [C, T], BF16, kind="ExternalInput")
    wq = nc.dram_tensor("wq", [C, 256], BF16, kind="ExternalInput")
    wk = nc.dram_tensor("wk", [C, 256], BF16, kind="ExternalInput")
    wv = nc.dram_tensor("wv", [C, 256], BF16, kind="ExternalInput")
    wo = nc.dram_tensor("wo", [256, C], BF16, kind="ExternalInput")
    cos2 = nc.dram_tensor("cos2", [128, T], F32, kind="ExternalInput")
    sin2 = nc.dram_tensor("sin2", [128, T], F32, kind="ExternalInput")
    tri = nc.dram_tensor("tri", [128, 128], BF16, kind="ExternalInput")
    if mode == "general":
        maskT = nc.dram_tensor("maskT", [T, T], BF16, kind="ExternalInput")
    y = nc.dram_tensor("out", [T, C], F32, kind="ExternalOutput")

    with TileContext(nc) as tc:
        with (
            tc.tile_pool(name="persist", bufs=1) as persist,
            tc.tile_pool(name="work", bufs=2) as work,
            tc.tile_pool(name="epool", bufs=3) as epool,
        ):
            # ---- persistent SBUF tensors (live through both phases) ----
            qT_sb = [persist.tile([128, T], BF16, name=f"qT{p}") for p in range(2)]
            kT_sb = [persist.tile([128, T], BF16, name=f"kT{p}") for p in range(2)]
            # v token-major with interleaved ones column per head: 4 x 65 cols
            v_sb = [persist.tile([128, HG * (Dh + 1)], BF16, name=f"v{j}")
                    for j in range(KT)]
            wo_sb = persist.tile([128, 2, C], BF16, name="wo_sb")
            tri_sb = persist.tile([128, 128], BF16, name="tri_sb")
            nc.sync.dma_start(wo_sb[:], wo.rearrange("(p2 p) n -> p p2 n", p=128))
            nc.sync.dma_start(tri_sb[:], tri[:])

            # ================= phase 1: projections + RoPE =================
            with (
                tc.tile_pool(name="xw", bufs=1) as xw,
                tc.tile_pool(name="ppsum", bufs=4, space="PSUM") as ppsum,
                tc.tile_pool(name="rope", bufs=3) as rope,
            ):
                x_sb = xw.tile([128, KC, T], BF16)
                xv = xT.rearrange("(kt p) t -> kt p t", p=128)
                for k in range(KC):
                    nc.sync.dma_start(x_sb[:, k, :], xv[k])
                w_sb = {}
                for nm, dram in (("q", wq), ("k", wk), ("v", wv)):
                    w_sb[nm] = xw.tile([128, KC, 256], BF16, name=f"w{nm}_sb")
                    nc.sync.dma_start(
                        w_sb[nm][:], dram.rearrange("(kt p) m -> p kt m", p=128))
                cos_sb = xw.tile([128, T], F32)
                sin_sb = xw.tile([128, T], F32)
                nc.sync.dma_start(cos_sb[:], cos2[:])
                nc.sync.dma_start(sin_sb[:], sin2[:])

                # q/k projections -> head-dim-major + fused RoPE
                for nm, dest in (("q", qT_sb), ("k", kT_sb)):
                    for p in range(2):          # head-pair tile (2 heads)
                        for n in range(NCHUNK):  # 512-token chunk
                            ps = ppsum.tile([128, 512], F32, tag="proj")
                            for k in range(KC):
                                nc.tensor.matmul(
                                    ps[:],
                                    w_sb[nm][:, k, ts(p, 128)],
                                    x_sb[:, k, ts(n, 512)],
                                    start=(k == 0), stop=(k == KC - 1))
                            # RoPE: out = ps*cos + swap32(ps * sin_preswapped)
                            t1 = rope.tile([128, 512], F32, tag="t1")
                            nc.vector.tensor_mul(
                                t1[:], ps[:], cos_sb[:, ts(n, 512)])
                            t2p = rope.tile([128, 512], F32, tag="t2p")
                            nc.vector.tensor_mul(
                                t2p[:], ps[:], sin_sb[:, ts(n, 512)])
                            t2 = rope.tile([128, 512], F32, tag="t2")
                            for a, bq in ((0, 32), (32, 0), (64, 96), (96, 64)):
                                nc.sync.dma_start(
                                    t2[a:a + 32, :], t2p[bq:bq + 32, :])
                            nc.vector.tensor_add(
                                dest[p][:, ts(n, 512)], t1[:], t2[:])

                # v projection: token-major with ones column per head
                for tt in range(T // 128):
                    ps = ppsum.tile([128, 256], F32, tag="vproj")
                    for k in range(KC):
                        nc.tensor.matmul(
                            ps[:],
                            x_sb[:, k, ts(tt, 128)],
                            w_sb["v"][:, k, :],
                            start=(k == 0), stop=(k == KC - 1))
                    vt = v_sb[tt].rearrange("p (h d) -> p h d", h=HG)
                    nc.vector.tensor_copy(vt[:, :, 0:Dh], ps.rearrange(
                        "p (h d) -> p h d", h=HG))
                    nc.vector.memset(vt[:, :, Dh:Dh + 1], 1.0)

            # ================= phase 2: attention + o-proj =================
            with (
                tc.tile_pool(name="apsum", bufs=2, space="PSUM") as apsum,
                tc.tile_pool(name="opool", bufs=2) as opool,
                tc.tile_pool(name="mpool", bufs=1) as mpool,
            ):
                if mode == "general":
                    msk_sb = mpool.tile([128, KT, T], BF16)
                    mv = maskT.rearrange("(kt p) t -> kt p t", p=128)
                    for j in range(KT):
                        nc.sync.dma_start(msk_sb[:, j, :], mv[j])

                for c in range(NCHUNK):
                    o_sb = [opool.tile([128, 512], BF16, tag=f"o{p}",
                                       name=f"o_sb{p}") for p in range(2)]
                    for h in range(HG):
                        p, off = h // 2, (h % 2) * 64
                        nkt = 4 * (c + 1) if mode == "causal" else KT
                        ngroups = (nkt + 1) // 2
                        psO = apsum.tile([128, 512], F32, tag="o")
                        for g in range(ngroups):
                            psS = apsum.tile([128, 1024], F32, tag="s")
                            E = epool.tile([128, 1024], BF16, tag="E")
                            slots = [j for j in (2 * g, 2 * g + 1) if j < nkt]
                            for s_i, j in enumerate(slots):
                                n_off = 0
                                if mode == "causal" and j >= 4 * c:
                                    n_off = 128 * (j - 4 * c)
                                nc.tensor.matmul(
                                    psS[:, ds(512 * s_i + n_off, 512 - n_off)],
                                    kT_sb[p][off:off + 64, ts(j, 128)],
                                    qT_sb[p][off:off + 64,
                                             ds(512 * c + n_off, 512 - n_off)],
                                    start=True, stop=True)
                            nc.scalar.activation(
                                E[:], psS[:],
                                mybir.ActivationFunctionType.Exp, scale=SCALE)
                            for s_i, j in enumerate(slots):
                                n_off = 0
                                if mode == "causal" and j >= 4 * c:
                                    n_off = 128 * (j - 4 * c)
                                    if n_off < 512:
                                        nc.vector.tensor_mul(
                                            E[:, ds(512 * s_i + n_off, 128)],
                                            E[:, ds(512 * s_i + n_off, 128)],
                                            tri_sb[:])
                                if mode == "general":
                                    nc.vector.tensor_mul(
                                        E[:, ts(s_i, 512)], E[:, ts(s_i, 512)],
                                        msk_sb[:, j, ts(c, 512)])
                                nc.tensor.matmul(
                                    psO[0:65, ds(n_off, 512 - n_off)],
                                    v_sb[j][:, ds(h * (Dh + 1), Dh + 1)],
                                    E[:, ds(512 * s_i + n_off, 512 - n_off)],
                                    start=(j == 0), stop=(j == nkt - 1))
                        # normalize: rows/denominator
                        r_sb = opool.tile([1, 512], F32, tag="r")
                        nc.vector.reciprocal(r_sb[:], psO[64:65, :])
                        rb_sb = opool.tile([64, 512], F32, tag="rb")
                        nc.gpsimd.partition_broadcast(rb_sb[:], r_sb[:])
                        nc.vector.tensor_mul(
                            o_sb[p][off:off + 64, :], psO[0:64, :], rb_sb[:])
                    # output projection for this chunk
                    for tt in range(4):
                        y_sb = opool.tile([128, C], F32, tag="y")
                        for nn in range(2):
                            psY = apsum.tile([128, 512], F32, tag="s")
                            for p in range(2):
                                nc.tensor.matmul(
                                    psY[:],
                                    o_sb[p][:, ts(tt, 128)],
                                    wo_sb[:, p, ts(nn, 512)],
                                    start=(p == 0), stop=(p == 1))
                            nc.vector.tensor_copy(y_sb[:, ts(nn, 512)], psY[:])
                        nc.sync.dma_start(y[ds(512 * c + 128 * tt, 128), :], y_sb[:])

    nc.finalize()
    return nc


_CACHE: dict = {}


def _get_nc(mode: str):
    if mode not in _CACHE:
        _CACHE[mode] = build(mode)
    return _CACHE[mode]


def kernel(x, cos, sin, mask, n_heads, Wq, Wk, Wv, Wo, _trace=False):
    _install_ntff_hook()
    x = np.asarray(x, np.float32)
    cos = np.asarray(cos, np.float32)
    sin = np.asarray(sin, np.float32)
    mask = np.asarray(mask)
    Wq = np.asarray(Wq, np.float32)
    Wk = np.asarray(Wk, np.float32)
    Wv = np.asarray(Wv, np.float32)
    Wo = np.asarray(Wo, np.float32)

    if np.array_equal(mask, np.tril(np.ones((T, T), mask.dtype))):
        mode = "causal"
    elif np.all(mask == 1):
        mode = "full"
    else:
        mode = "general"

    cosT = np.ascontiguousarray(cos.T)          # (64, T)
    # pre-swapped signed sin: after multiplying q by this and swapping the
    # 32-row halves, we get rotate_half(q)*sin in standard orientation.
    sinS = np.ascontiguousarray(sin.T).copy()
    sinS[32:64] *= -1.0
    cos2 = np.vstack([cosT, cosT])              # (128, T)
    sin2 = np.vstack([sinS, sinS])
    tri = np.triu(np.ones((128, 128), np.float32)).astype(ml_dtypes.bfloat16)

    in_maps = []
    for core in range(NCORES):
        b, g = core // 4, core % 4
        rows = slice(g * 256, (g + 1) * 256)
        m = {
            "xT": np.ascontiguousarray(x[b].T).astype(ml_dtypes.bfloat16),
            "wq": np.ascontiguousarray(Wq[rows].T).astype(ml_dtypes.bfloat16),
            "wk": np.ascontiguousarray(Wk[rows].T).astype(ml_dtypes.bfloat16),
            "wv": np.ascontiguousarray(Wv[rows].T).astype(ml_dtypes.bfloat16),
            "wo": np.ascontiguousarray(Wo[:, rows].T).astype(ml_dtypes.bfloat16),
            "cos2": cos2, "sin2": sin2, "tri": tri,
        }
        if mode == "general":
            m["maskT"] = np.ascontiguousarray(mask.T).astype(ml_dtypes.bfloat16)
        in_maps.append(m)

    nc = _get_nc(mode)
    res = bass_utils.run_bass_kernel_spmd(
        nc, in_maps, core_ids=list(range(NCORES)), trace=_trace)
    if _trace:
        kernel.last_result = res

    y = np.zeros((B, T, C), np.float32)
    for core in range(NCORES):
        y[core // 4] += res.results[core]["out"]
    return y


# revision 9
# speedup vs baseline: 1.8394x; 1.1227x over previous
"""Distributed Trainium2 Bass kernel for causal multi-head attention.

Problem: B=2, T=2048, C=1024, H=16 heads (Dh=64), RoPE + causal mask +
softmax + output projection.

Sharding: 8 cores = batch (2) x head-groups (4 heads each). Each core
computes q/k/v projections for its 4 heads, RoPE, attention, and a
partial output projection y_partial = out_heads @ Wo_slice.T. The host
sums the 4 partials per batch element.

Layout trick: everything is computed in "head-dim-major" (transposed)
layout so no on-chip transposes are needed:
  qT/kT: (dims, tokens) from projection matmuls directly
  S^T = K @ Q^T tiles (keys, tokens): softmax denominator via an
    appended ones-column in V (extra row of PV output = sum over keys)
  PV: O'^T = V_aug^T @ P^T -> (65, toks) in PSUM, row 64 = denominator
  o-proj consumes O^T directly as the stationary operand.
"""

import os
import sys
import types
import numpy as np

sys.path.insert(0, "/opt/trn_rl_repo")

import ml_dtypes
import concourse.bass as bass
import concourse.mybir as mybir
from concourse import bacc
from concourse.tile import TileContext
from concourse import bass_utils
from concourse.bass import ts, ds

F32 = mybir.dt.float32
BF16 = mybir.dt.bfloat16

B, T, C, H = 2, 2048, 1024, 16
Dh = C // H          # 64
HG = 4               # heads per core
NCORES = 8
KC = C // 128        # 8 contraction tiles for projections
NCHUNK = T // 512    # 4 token chunks
KT = T // 128        # 16 key tiles
SCALE = Dh ** -0.5   # 0.125


def _install_ntff_hook():
    """The NTFF profiling hook module is absent in this image; inject it."""
    if "antenv.axon_hooks" in sys.modules:
        return
    try:
        import trn_agent_boot.trn_boot as tb
        mod = types.ModuleType("antenv.axon_hooks")
        hook = tb._ntff_profile_via_ctypes("/opt/axon/libaxon_pjrt.so")
        mod.get_axon_ntff_profile_hook = lambda: hook
        sys.modules["antenv.axon_hooks"] = mod
    except Exception:
        pass


def build(mode: str) -> bass.Bass:
    """mode: 'causal' | 'full' | 'general'"""
    assert mode in ("causal", "full", "general")
    nc = bacc.Bacc(None, target_bir_lowering=False)

    xT = nc.dram_tensor("xT", [C, T], BF16, kind="ExternalInput")
    wq = nc.dram_tensor("wq", [C, 256], BF16, kind="ExternalInput")
    wk = nc.dram_tensor("wk", [C, 256], BF16, kind="ExternalInput")
    wv = nc.dram_tensor("wv", [C, 256], BF16, kind="ExternalInput")
    wo = nc.dram_tensor("wo", [256, C], BF16, kind="ExternalInput")
    cos2 = nc.dram_tensor("cos2", [128, T], F32, kind="ExternalInput")
    sin2 = nc.dram_tensor("sin2", [128, T], F32, kind="ExternalInput")
    tri = nc.dram_tensor("tri", [128, 128], BF16, kind="ExternalInput")
    if mode == "general":
        maskT = nc.dram_tensor("maskT", [T, T], BF16, kind="ExternalInput")
    y = nc.dram_tensor("out", [T, C], F32, kind="ExternalOutput")

    with TileContext(nc) as tc:
        with (
            tc.tile_pool(name="persist", bufs=1) as persist,
            tc.tile_pool(name="work", bufs=2) as work,
            tc.tile_pool(name="epool", bufs=3) as epool,
        ):
            # ---- persistent SBUF tensors (live through both phases) ----
            # q is stored per-head in full 128-partition tiles with the other
            # head's rows zeroed: matmuls with base-partition-64 operands run
            # ~7.5x slower on TRN2, so QK contracts over the full 128 dims of
            # the k head-pair tile and the zeros kill the cross-head terms.
            qT_sb = [persist.tile([128, T], BF16, name=f"qT{h}") for h in range(HG)]
            kT_sb = [persist.tile([128, T], BF16, name=f"kT{p}") for p in range(2)]
            for h in range(HG):
                off = (h % 2) * 64
                nc.vector.memset(qT_sb[h][64 - off:128 - off, :], 0.0)
            # v token-major with interleaved ones column per head: 4 x 65 cols
            v_sb = [persist.tile([128, HG * (Dh + 1)], BF16, name=f"v{j}")
                    for j in range(KT)]
            wo_sb = persist.tile([128, 2, C], BF16, name="wo_sb")
            tri_sb = persist.tile([128, 128], BF16, name="tri_sb")
            nc.sync.dma_start(wo_sb[:], wo.rearrange("(p2 p) n -> p p2 n", p=128))
            nc.sync.dma_start(tri_sb[:], tri[:])

            # ================= phase 1: projections + RoPE =================
            with (
                tc.tile_pool(name="xw", bufs=1) as xw,
                tc.tile_pool(name="ppsum", bufs=4, space="PSUM") as ppsum,
                tc.tile_pool(name="rope", bufs=3) as rope,
            ):
                x_sb = xw.tile([128, KC, T], BF16)
                xv = xT.rearrange("(kt p) t -> kt p t", p=128)
                for k in range(KC):
                    nc.sync.dma_start(x_sb[:, k, :], xv[k])
                w_sb = {}
                for nm, dram in (("q", wq), ("k", wk), ("v", wv)):
                    w_sb[nm] = xw.tile([128, KC, 256], BF16, name=f"w{nm}_sb")
                    nc.sync.dma_start(
                        w_sb[nm][:], dram.rearrange("(kt p) m -> p kt m", p=128))
                cos_sb = xw.tile([128, T], F32)
                sin_sb = xw.tile([128, T], F32)
                nc.sync.dma_start(cos_sb[:], cos2[:])
                nc.sync.dma_start(sin_sb[:], sin2[:])

                # q/k projections -> head-dim-major + fused RoPE
                for nm, dest in (("q", qT_sb), ("k", kT_sb)):
                    for p in range(2):          # head-pair tile (2 heads)
                        for n in range(NCHUNK):  # 512-token chunk
                            ps = ppsum.tile([128, 512], F32, tag="proj")
                            for k in range(KC):
                                nc.tensor.matmul(
                                    ps[:],
                                    w_sb[nm][:, k, ts(p, 128)],
                                    x_sb[:, k, ts(n, 512)],
                                    start=(k == 0), stop=(k == KC - 1))
                            # RoPE: out = ps*cos + swap32(ps * sin_preswapped)
                            t1 = rope.tile([128, 512], F32, tag="t1")
                            nc.vector.tensor_mul(
                                t1[:], ps[:], cos_sb[:, ts(n, 512)])
                            t2p = rope.tile([128, 512], F32, tag="t2p")
                            nc.vector.tensor_mul(
                                t2p[:], ps[:], sin_sb[:, ts(n, 512)])
                            t2 = rope.tile([128, 512], F32, tag="t2")
                            for a, bq in ((0, 32), (32, 0), (64, 96), (96, 64)):
                                nc.sync.dma_start(
                                    t2[a:a + 32, :], t2p[bq:bq + 32, :])
                            if nm == "q":
                                # per-head zero-padded tiles
                                for hh in range(2):
                                    o = hh * 64
                                    nc.vector.tensor_add(
                                        dest[2 * p + hh][o:o + 64, ts(n, 512)],
                                        t1[o:o + 64, :], t2[o:o + 64, :])
                            else:
                                nc.vector.tensor_add(
                                    dest[p][:, ts(n, 512)], t1[:], t2[:])

                # v projection: token-major with ones column per head
                for tt in range(T // 128):
                    ps = ppsum.tile([128, 256], F32, tag="vproj")
                    for k in range(KC):
                        nc.tensor.matmul(
                            ps[:],
                            x_sb[:, k, ts(tt, 128)],
                            w_sb["v"][:, k, :],
                            start=(k == 0), stop=(k == KC - 1))
                    vt = v_sb[tt].rearrange("p (h d) -> p h d", h=HG)
                    nc.vector.tensor_copy(vt[:, :, 0:Dh], ps.rearrange(
                        "p (h d) -> p h d", h=HG))
                    nc.vector.memset(vt[:, :, Dh:Dh + 1], 1.0)

            # ================= phase 2: attention + o-proj =================
            with (
                tc.tile_pool(name="apsum", bufs=2, space="PSUM") as apsum,
                tc.tile_pool(name="opool", bufs=2) as opool,
                tc.tile_pool(name="mpool", bufs=1) as mpool,
            ):
                if mode == "general":
                    msk_sb = mpool.tile([128, KT, T], BF16)
                    mv = maskT.rearrange("(kt p) t -> kt p t", p=128)
                    for j in range(KT):
                        nc.sync.dma_start(msk_sb[:, j, :], mv[j])

                for c in range(NCHUNK):
                    o_sb = [opool.tile([128, 512], BF16, tag=f"o{p}",
                                       name=f"o_sb{p}") for p in range(2)]
                    for h in range(HG):
                        p, off = h // 2, (h % 2) * 64
                        nkt = 4 * (c + 1) if mode == "causal" else KT
                        ngroups = (nkt + 1) // 2
                        psO = apsum.tile([128, 512], F32, tag="o")
                        for g in range(ngroups):
                            psS = apsum.tile([128, 1024], F32, tag="s")
                            E = epool.tile([128, 1024], BF16, tag="E")
                            slots = [j for j in (2 * g, 2 * g + 1) if j < nkt]
                            for s_i, j in enumerate(slots):
                                n_off = 0
                                if mode == "causal" and j >= 4 * c:
                                    n_off = 128 * (j - 4 * c)
                                nc.tensor.matmul(
                                    psS[:, ds(512 * s_i + n_off, 512 - n_off)],
                                    kT_sb[p][:, ts(j, 128)],
                                    qT_sb[h][:, ds(512 * c + n_off, 512 - n_off)],
                                    start=True, stop=True)
                            nc.scalar.activation(
                                E[:], psS[:],
                                mybir.ActivationFunctionType.Exp, scale=SCALE)
                            for s_i, j in enumerate(slots):
                                n_off = 0
                                if mode == "causal" and j >= 4 * c:
                                    n_off = 128 * (j - 4 * c)
                                    if n_off < 512:
                                        nc.vector.tensor_mul(
                                            E[:, ds(512 * s_i + n_off, 128)],
                                            E[:, ds(512 * s_i + n_off, 128)],
                                            tri_sb[:])
                                if mode == "general":
                                    nc.vector.tensor_mul(
                                        E[:, ts(s_i, 512)], E[:, ts(s_i, 512)],
                                        msk_sb[:, j, ts(c, 512)])
                                nc.tensor.matmul(
                                    psO[0:65, ds(n_off, 512 - n_off)],
                                    v_sb[j][:, ds(h * (Dh + 1), Dh + 1)],
                                    E[:, ds(512 * s_i + n_off, 512 - n_off)],
                                    start=(j == 0), stop=(j == nkt - 1))
                        # normalize: rows/denominator
                        r_sb = opool.tile([1, 512], F32, tag="r")
                        nc.vector.reciprocal(r_sb[:], psO[64:65, :])
                        rb_sb = opool.tile([64, 512], F32, tag="rb")
                        nc.gpsimd.partition_broadcast(rb_sb[:], r_sb[:])
                        nc.vector.tensor_mul(
                            o_sb[p][off:off + 64, :], psO[0:64, :], rb_sb[:])
                    # output projection for this chunk
                    for tt in range(4):
                        y_sb = opool.tile([128, C], F32, tag="y")
                        for nn in range(2):
                            psY = apsum.tile([128, 512], F32, tag="s")
                            for p in range(2):
                                nc.tensor.matmul(
                                    psY[:],
                                    o_sb[p][:, ts(tt, 128)],
                                    wo_sb[:, p, ts(nn, 512)],
                                    start=(p == 0), stop=(p == 1))
                            nc.vector.tensor_copy(y_sb[:, ts(nn, 512)], psY[:])
                        nc.sync.dma_start(y[ds(512 * c + 128 * tt, 128), :], y_sb[:])

    nc.finalize()
    return nc


_CACHE: dict = {}


def _get_nc(mode: str):
    if mode not in _CACHE:
        _CACHE[mode] = build(mode)
    return _CACHE[mode]


def kernel(x, cos, sin, mask, n_heads, Wq, Wk, Wv, Wo, _trace=False):
    _install_ntff_hook()
    x = np.asarray(x, np.float32)
    cos = np.asarray(cos, np.float32)
    sin = np.asarray(sin, np.float32)
    mask = np.asarray(mask)
    Wq = np.asarray(Wq, np.float32)
    Wk = np.asarray(Wk, np.float32)
    Wv = np.asarray(Wv, np.float32)
    Wo = np.asarray(Wo, np.float32)

    if np.array_equal(mask, np.tril(np.ones((T, T), mask.dtype))):
        mode = "causal"
    elif np.all(mask == 1):
        mode = "full"
    else:
        mode = "general"

    cosT = np.ascontiguousarray(cos.T)          # (64, T)
    # pre-swapped signed sin: after multiplying q by this and swapping the
    # 32-row halves, we get rotate_half(q)*sin in standard orientation.
    sinS = np.ascontiguousarray(sin.T).copy()
    sinS[32:64] *= -1.0
    cos2 = np.vstack([cosT, cosT])              # (128, T)
    sin2 = np.vstack([sinS, sinS])
    tri = np.triu(np.ones((128, 128), np.float32)).astype(ml_dtypes.bfloat16)

    in_maps = []
    for core in range(NCORES):
        b, g = core // 4, core % 4
        rows = slice(g * 256, (g + 1) * 256)
        m = {
            "xT": np.ascontiguousarray(x[b].T).astype(ml_dtypes.bfloat16),
            "wq": np.ascontiguousarray(Wq[rows].T).astype(ml_dtypes.bfloat16),
            "wk": np.ascontiguousarray(Wk[rows].T).astype(ml_dtypes.bfloat16),
            "wv": np.ascontiguousarray(Wv[rows].T).astype(ml_dtypes.bfloat16),
            "wo": np.ascontiguousarray(Wo[:, rows].T).astype(ml_dtypes.bfloat16),
            "cos2": cos2, "sin2": sin2, "tri": tri,
        }
        if mode == "general":
            m["maskT"] = np.ascontiguousarray(mask.T).astype(ml_dtypes.bfloat16)
        in_maps.append(m)

    nc = _get_nc(mode)
    res = bass_utils.run_bass_kernel_spmd(
        nc, in_maps, core_ids=list(range(NCORES)), trace=_trace)
    if _trace:
        kernel.last_result = res

    y = np.zeros((B, T, C), np.float32)
    for core in range(NCORES):
        y[core // 4] += res.results[core]["out"]
    return y


# revision 13
# speedup vs baseline: 1.9480x; 1.0590x over previous
"""Distributed Trainium2 Bass kernel for causal multi-head attention.

Problem: B=2, T=2048, C=1024, H=16 heads (Dh=64), RoPE + causal mask +
softmax + output projection.

Sharding: 8 cores = batch (2) x head-groups (4 heads each). Each core
computes q/k/v projections for its 4 heads, RoPE, attention, and a
partial output projection y_partial = out_heads @ Wo_slice.T. The host
sums the 4 partials per batch element.

Layout trick: everything is computed in "head-dim-major" (transposed)
layout so no on-chip transposes are needed:
  qT/kT: (dims, tokens) from projection matmuls directly
  S^T = K @ Q^T tiles (keys, tokens): softmax denominator via an
    appended ones-column in V (extra row of PV output = sum over keys)
  PV: O'^T = V_aug^T @ P^T -> (65, toks) in PSUM, row 64 = denominator
  o-proj consumes O^T directly as the stationary operand.
"""

import os
import sys
import types
import numpy as np

sys.path.insert(0, "/opt/trn_rl_repo")

import ml_dtypes
import concourse.bass as bass
import concourse.mybir as mybir
from concourse import bacc
from concourse.tile import TileContext
from concourse import bass_utils
from concourse.bass import ts, ds

F32 = mybir.dt.float32
BF16 = mybir.dt.bfloat16

B, T, C, H = 2, 2048, 1024, 16
Dh = C // H          # 64
HG = 4               # heads per core
NCORES = 8
KC = C // 128        # 8 contraction tiles for projections
NCHUNK = T // 512    # 4 token chunks
KT = T // 128        # 16 key tiles
SCALE = Dh ** -0.5   # 0.125


def _install_ntff_hook():
    """The NTFF profiling hook module is absent in this image; inject it."""
    if "antenv.axon_hooks" in sys.modules:
        return
    try:
        import trn_agent_boot.trn_boot as tb
        mod = types.ModuleType("antenv.axon_hooks")
        hook = tb._ntff_profile_via_ctypes("/opt/axon/libaxon_pjrt.so")
        mod.get_axon_ntff_profile_hook = lambda: hook
        sys.modules["antenv.axon_hooks"] = mod
    except Exception:
        pass


def build(mode: str) -> bass.Bass:
    """mode: 'causal' | 'full' | 'general'"""
    assert mode in ("causal", "full", "general")
    nc = bacc.Bacc(None, target_bir_lowering=False)

    xT = nc.dram_tensor("xT", [C, T], BF16, kind="ExternalInput")
    wq = nc.dram_tensor("wq", [C, 256], BF16, kind="ExternalInput")
    wk = nc.dram_tensor("wk", [C, 256], BF16, kind="ExternalInput")
    wv = nc.dram_tensor("wv", [C, 256], BF16, kind="ExternalInput")
    wo = nc.dram_tensor("wo", [256, C], BF16, kind="ExternalInput")
    cos2 = nc.dram_tensor("cos2", [128, T], F32, kind="ExternalInput")
    sin2 = nc.dram_tensor("sin2", [128, T], F32, kind="ExternalInput")
    tri = nc.dram_tensor("tri", [128, 128], BF16, kind="ExternalInput")
    if mode == "general":
        maskT = nc.dram_tensor("maskT", [T, T], BF16, kind="ExternalInput")
    y = nc.dram_tensor("out", [T, C], F32, kind="ExternalOutput")

    with TileContext(nc) as tc:
        with (
            tc.tile_pool(name="persist", bufs=1) as persist,
            tc.tile_pool(name="work", bufs=2) as work,
            tc.tile_pool(name="epool", bufs=4) as epool,
        ):
            # ---- persistent SBUF tensors (live through both phases) ----
            # q is stored per-head in full 128-partition tiles with the other
            # head's rows zeroed: matmuls with base-partition-64 operands run
            # ~7.5x slower on TRN2, so QK contracts over the full 128 dims of
            # the k head-pair tile and the zeros kill the cross-head terms.
            qT_sb = [persist.tile([128, T], BF16, name=f"qT{h}") for h in range(HG)]
            kT_sb = [persist.tile([128, T], BF16, name=f"kT{p}") for p in range(2)]
            for h in range(HG):
                off = (h % 2) * 64
                nc.vector.memset(qT_sb[h][64 - off:128 - off, :], 0.0)
            # v token-major with interleaved ones column per head: 4 x 65 cols
            v_sb = [persist.tile([128, HG * (Dh + 1)], BF16, name=f"v{j}")
                    for j in range(KT)]
            wo_sb = persist.tile([128, 2, C], BF16, name="wo_sb")
            tri_sb = persist.tile([128, 128], BF16, name="tri_sb")
            nc.sync.dma_start(wo_sb[:], wo.rearrange("(p2 p) n -> p p2 n", p=128))
            nc.sync.dma_start(tri_sb[:], tri[:])

            # ================= phase 1: projections + RoPE =================
            with (
                tc.tile_pool(name="xw", bufs=1) as xw,
                tc.tile_pool(name="ppsum", bufs=4, space="PSUM") as ppsum,
                tc.tile_pool(name="rope", bufs=3) as rope,
            ):
                x_sb = xw.tile([128, KC, T], BF16)
                xv = xT.rearrange("(kt p) t -> kt p t", p=128)
                for k in range(KC):
                    nc.sync.dma_start(x_sb[:, k, :], xv[k])
                w_sb = {}
                for nm, dram in (("q", wq), ("k", wk), ("v", wv)):
                    w_sb[nm] = xw.tile([128, KC, 256], BF16, name=f"w{nm}_sb")
                    nc.sync.dma_start(
                        w_sb[nm][:], dram.rearrange("(kt p) m -> p kt m", p=128))
                cos_sb = xw.tile([128, T], F32)
                sin_sb = xw.tile([128, T], F32)
                nc.sync.dma_start(cos_sb[:], cos2[:])
                nc.sync.dma_start(sin_sb[:], sin2[:])

                # q/k projections -> head-dim-major + fused RoPE
                for nm, dest in (("q", qT_sb), ("k", kT_sb)):
                    for p in range(2):          # head-pair tile (2 heads)
                        for n in range(NCHUNK):  # 512-token chunk
                            ps = ppsum.tile([128, 512], F32, tag="proj")
                            for k in range(KC):
                                nc.tensor.matmul(
                                    ps[:],
                                    w_sb[nm][:, k, ts(p, 128)],
                                    x_sb[:, k, ts(n, 512)],
                                    start=(k == 0), stop=(k == KC - 1))
                            # RoPE: out = ps*cos + swap32(ps * sin_preswapped)
                            t1 = rope.tile([128, 512], F32, tag="t1")
                            nc.vector.tensor_mul(
                                t1[:], ps[:], cos_sb[:, ts(n, 512)])
                            t2p = rope.tile([128, 512], F32, tag="t2p")
                            nc.vector.tensor_mul(
                                t2p[:], ps[:], sin_sb[:, ts(n, 512)])
                            t2 = rope.tile([128, 512], F32, tag="t2")
                            for a, bq in ((0, 32), (32, 0), (64, 96), (96, 64)):
                                nc.sync.dma_start(
                                    t2[a:a + 32, :], t2p[bq:bq + 32, :])
                            if nm == "q":
                                # per-head zero-padded tiles
                                for hh in range(2):
                                    o = hh * 64
                                    nc.gpsimd.tensor_add(
                                        dest[2 * p + hh][o:o + 64, ts(n, 512)],
                                        t1[o:o + 64, :], t2[o:o + 64, :])
                            else:
                                nc.gpsimd.tensor_add(
                                    dest[p][:, ts(n, 512)], t1[:], t2[:])

                # v projection: token-major with ones column per head
                for tt in range(T // 128):
                    ps = ppsum.tile([128, 256], F32, tag="vproj")
                    for k in range(KC):
                        nc.tensor.matmul(
                            ps[:],
                            x_sb[:, k, ts(tt, 128)],
                            w_sb["v"][:, k, :],
                            start=(k == 0), stop=(k == KC - 1))
                    vt = v_sb[tt].rearrange("p (h d) -> p h d", h=HG)
                    nc.scalar.copy(vt[:, :, 0:Dh], ps.rearrange(
                        "p (h d) -> p h d", h=HG))
                    nc.vector.memset(vt[:, :, Dh:Dh + 1], 1.0)

            # ================= phase 2: attention + o-proj =================
            with (
                tc.tile_pool(name="apsum", bufs=2, space="PSUM") as apsum,
                tc.tile_pool(name="opool", bufs=2) as opool,
                tc.tile_pool(name="mpool", bufs=1) as mpool,
            ):
                if mode == "general":
                    msk_sb = mpool.tile([128, KT, T], BF16)
                    mv = maskT.rearrange("(kt p) t -> kt p t", p=128)
                    for j in range(KT):
                        nc.sync.dma_start(msk_sb[:, j, :], mv[j])

                def emit_oproj(c, o_sb):
                    for tt in range(4):
                        y_sb = opool.tile([128, C], F32, tag="y", name="y_sb")
                        for nn in range(2):
                            psY = apsum.tile([128, 512], F32, tag="y",
                                             name="psY")
                            for p in range(2):
                                nc.tensor.matmul(
                                    psY[:],
                                    o_sb[p][:, ts(tt, 128)],
                                    wo_sb[:, p, ts(nn, 512)],
                                    start=(p == 0), stop=(p == 1))
                            nc.vector.tensor_copy(y_sb[:, ts(nn, 512)], psY[:])
                        nc.sync.dma_start(y[ds(512 * c + 128 * tt, 128), :], y_sb[:])

                pending = None  # (chunk_idx, o_sb) awaiting output projection
                for c in range(NCHUNK):
                    o_sb = [opool.tile([128, 512], BF16, tag=f"o{p}",
                                       name=f"o_sb{p}") for p in range(2)]
                    nkt = 4 * (c + 1) if mode == "causal" else KT
                    ngroups = (nkt + 1) // 2
                    for hp in range(2):           # head pair
                        psO = {}
                        for hh in range(2):
                            h = 2 * hp + hh
                            psO[h] = apsum.tile([128, 512], F32, tag="o",
                                                name=f"psO{hh}")
                        for g in range(ngroups):
                            slots = [j for j in (2 * g, 2 * g + 1) if j < nkt]
                            Es = {}
                            for hh in range(2):   # interleave the two heads
                                h = 2 * hp + hh
                                psS = apsum.tile([128, 1024], F32, tag="s",
                                                 name="psS")
                                E = epool.tile([128, 1024], BF16, tag="E",
                                               name="E")
                                Es[h] = E
                                for s_i, j in enumerate(slots):
                                    n_off = 0
                                    if mode == "causal" and j >= 4 * c:
                                        n_off = 128 * (j - 4 * c)
                                    nc.tensor.matmul(
                                        psS[:, ds(512 * s_i + n_off, 512 - n_off)],
                                        kT_sb[hp][:, ts(j, 128)],
                                        qT_sb[h][:, ds(512 * c + n_off,
                                                       512 - n_off)],
                                        start=True, stop=True)
                                nc.scalar.activation(
                                    E[:], psS[:],
                                    mybir.ActivationFunctionType.Exp,
                                    scale=SCALE)
                                for s_i, j in enumerate(slots):
                                    if mode == "causal" and j >= 4 * c:
                                        n_off = 128 * (j - 4 * c)
                                        if n_off < 512:
                                            nc.vector.tensor_mul(
                                                E[:, ds(512 * s_i + n_off, 128)],
                                                E[:, ds(512 * s_i + n_off, 128)],
                                                tri_sb[:])
                                    if mode == "general":
                                        nc.vector.tensor_mul(
                                            E[:, ts(s_i, 512)],
                                            E[:, ts(s_i, 512)],
                                            msk_sb[:, j, ts(c, 512)])
                            for hh in range(2):
                                h = 2 * hp + hh
                                for s_i, j in enumerate(slots):
                                    n_off = 0
                                    if mode == "causal" and j >= 4 * c:
                                        n_off = 128 * (j - 4 * c)
                                    nc.tensor.matmul(
                                        psO[h][0:65, ds(n_off, 512 - n_off)],
                                        v_sb[j][:, ds(h * (Dh + 1), Dh + 1)],
                                        Es[h][:, ds(512 * s_i + n_off,
                                                    512 - n_off)],
                                        start=(j == 0), stop=(j == nkt - 1))
                            if pending is not None and hp == 0 and g == 0:
                                # previous chunk's o-proj: fills the PE while
                                # this chunk's first exp/PV latencies resolve
                                emit_oproj(*pending)
                                pending = None
                        for hh in range(2):
                            h = 2 * hp + hh
                            off = hh * 64
                            r_sb = opool.tile([1, 512], F32, tag="r",
                                              name="r_sb")
                            nc.vector.reciprocal(r_sb[:], psO[h][64:65, :])
                            rb_sb = opool.tile([64, 512], F32, tag="rb",
                                               name="rb_sb")
                            nc.gpsimd.partition_broadcast(rb_sb[:], r_sb[:])
                            nc.vector.tensor_mul(
                                o_sb[hp][off:off + 64, :], psO[h][0:64, :],
                                rb_sb[:])
                    pending = (c, o_sb)
                emit_oproj(*pending)

    nc.finalize()
    return nc


_CACHE: dict = {}


def _get_nc(mode: str):
    if mode not in _CACHE:
        _CACHE[mode] = build(mode)
    return _CACHE[mode]


def kernel(x, cos, sin, mask, n_heads, Wq, Wk, Wv, Wo, _trace=False):
    _install_ntff_hook()
    x = np.asarray(x, np.float32)
    cos = np.asarray(cos, np.float32)
    sin = np.asarray(sin, np.float32)
    mask = np.asarray(mask)
    Wq = np.asarray(Wq, np.float32)
    Wk = np.asarray(Wk, np.float32)
    Wv = np.asarray(Wv, np.float32)
    Wo = np.asarray(Wo, np.float32)

    if np.array_equal(mask, np.tril(np.ones((T, T), mask.dtype))):
        mode = "causal"
    elif np.all(mask == 1):
        mode = "full"
    else:
        mode = "general"

    cosT = np.ascontiguousarray(cos.T)          # (64, T)
    # pre-swapped signed sin: after multiplying q by this and swapping the
    # 32-row halves, we get rotate_half(q)*sin in standard orientation.
    sinS = np.ascontiguousarray(sin.T).copy()
    sinS[32:64] *= -1.0
    cos2 = np.vstack([cosT, cosT])              # (128, T)
    sin2 = np.vstack([sinS, sinS])
    tri = np.triu(np.ones((128, 128), np.float32)).astype(ml_dtypes.bfloat16)

    in_maps = []
    for core in range(NCORES):
        b, g = core // 4, core % 4
        rows = slice(g * 256, (g + 1) * 256)
        m = {
            "xT": np.ascontiguousarray(x[b].T).astype(ml_dtypes.bfloat16),
            "wq": np.ascontiguousarray(Wq[rows].T).astype(ml_dtypes.bfloat16),
            "wk": np.ascontiguousarray(Wk[rows].T).astype(ml_dtypes.bfloat16),
            "wv": np.ascontiguousarray(Wv[rows].T).astype(ml_dtypes.bfloat16),
            "wo": np.ascontiguousarray(Wo[:, rows].T).astype(ml_dtypes.bfloat16),
            "cos2": cos2, "sin2": sin2, "tri": tri,
        }
        if mode == "general":
            m["maskT"] = np.ascontiguousarray(mask.T).astype(ml_dtypes.bfloat16)
        in_maps.append(m)

    nc = _get_nc(mode)
    res = bass_utils.run_bass_kernel_spmd(
        nc, in_maps, core_ids=list(range(NCORES)), trace=_trace)
    if _trace:
        kernel.last_result = res

    y = np.zeros((B, T, C), np.float32)
    for core in range(NCORES):
        y[core // 4] += res.results[core]["out"]
    return y
